# revision 10
# baseline (speedup 1.0000x reference)
"""GatedAttention Trainium2 kernel (8 NeuronCores, tensor-parallel over (batch, head-group)).

Sharding: core c handles batch b=c//4 and heads 4*(c%4)..4*(c%4)+3.
Each core computes qkv/gate projections for its heads from x[b], per-head
QK-RMS-norm + softmax attention + sigmoid gating, and a row-split o_proj
partial [S, D]. Host sums the 4 partials per batch and adds the residual.

Math notes:
- prenorm RMS scale r[s] cancels inside q/k RMS-norm, so q/k use raw x;
  r is only applied to the v/gate path (host-precomputed, fused into the
  v/gate PSUM->SBUF copyback scale).
- prenorm_w is folded into the projection weights on host.
- softmax runs without max-subtraction (scores are QK-normalized, |s|<~6).
- attention 1/sqrt(64) scale and q-norm are folded into rq = 1/sqrt(sumsq+64*eps).
- v carries an extra all-ones column so attn@v also yields the softmax sums.
- matmuls run in bf16 (fp32 PSUM accumulation); softmax/norm math in fp32.
"""

import json

import numpy as np
import ml_dtypes

import concourse.bass as bass
import concourse.bass_utils as bass_utils
import concourse.bass2jax as bass2jax
import concourse.mybir as mybir
import concourse.tile as tile
from concourse.tile import TileContext
from concourse.masks import make_identity
from concourse.vector_clock import ScopedClock, VectorClock

F32 = mybir.dt.float32
BF16 = mybir.dt.bfloat16
AF = mybir.ActivationFunctionType
BF = ml_dtypes.bfloat16

B, S, D = 2, 2048, 1024
NH_TOT, HD = 16, 64
NH = 4            # heads per core
EPS = 1e-5
P = 128
ST = S // P       # 16 s-tiles
KT = D // P       # 8 d-tiles
NCH = S // 512    # 4 sq chunks

# ----------------------------------------------------------------------------
# compat patches: this walrus build accepts only ONE sync-wait per instruction
# ----------------------------------------------------------------------------

def _patched_drain_and_barrier(self, tick_clock, wait_clock):
    nc = self.nc
    gc = tick_clock.global_clock
    n = len(gc)
    for p in range(n):
        t = gc[p]
        if t <= 0:
            continue
        vec = VectorClock([0] * n)
        vec.require_at_least(p, t)
        nop = nc.sync.nop(nofuse=True, hint=f"drain_wait_p{p}")
        wait_clock.add_sem_waits(nop.ins, ScopedClock({None: vec}))
    nc.sync.drain(fusable=False)
    nc.all_engine_barrier()
    assert self.sems is not None
    popped = nc._tile_sem_poison_stack.pop()
    assert popped is self._sem_poison
    nc.clear_and_free_semaphores(list(self.sems.allocated().values()))
    nc.all_engine_barrier()


def _split_multi_waits(bir_json: bytes) -> bytes:
    bj = json.loads(bir_json)
    n_split = 0
    for fn in bj.get("functions", []):
        for blk in fn.get("blocks", []):
            out = []
            for inst in blk.get("instructions", []):
                si = inst.get("sync_info")
                waits = si.get("on_wait", []) if si else []
                if len(waits) > 1:
                    for i, w in enumerate(waits[:-1]):
                        out.append({
                            "debug": inst.get("debug"),
                            "engine": inst["engine"],
                            "ins": [], "outs": [],
                            "name": f"{inst['name']}_sw{i}",
                            "opcode": "NoOp",
                            "sync_info": {"on_update": [], "on_wait": [w]},
                            "text_hint": "split_wait",
                        })
                        n_split += 1
                    si["on_wait"] = [waits[-1]]
                out.append(inst)
            blk["instructions"] = out
    if n_split:
        return json.dumps(bj).encode()
    return bir_json


_ORIG_COMPILE = bass_utils.compile_bir_kernel


def _patched_compile_bir_kernel(bir_json, tmpdir, neff_name="file.neff"):
    return _ORIG_COMPILE(_split_multi_waits(bir_json), tmpdir, neff_name)


def _apply_compat():
    tile.TileContext._drain_and_barrier = _patched_drain_and_barrier
    bass_utils.compile_bir_kernel = _patched_compile_bir_kernel
    bass2jax.compile_bir_kernel = _patched_compile_bir_kernel


_apply_compat()

# ----------------------------------------------------------------------------
# device program (SPMD: identical program, per-core data)
# ----------------------------------------------------------------------------

_NC_CACHE = None


def _build_program():
    nc = bass.Bass()
    xt = nc.declare_dram_parameter("xt", [P, KT, S], BF16, isOutput=False)
    wqk = nc.declare_dram_parameter("wqk", [P, KT, 512], BF16, isOutput=False)
    wvg = nc.declare_dram_parameter("wvg", [P, KT, 260], BF16, isOutput=False)
    wo = nc.declare_dram_parameter("wo", [64, NH, D], BF16, isOutput=False)
    rv = nc.declare_dram_parameter("rv", [P, ST], F32, isOutput=False)
    qn = nc.declare_dram_parameter("qn", [P, 1], F32, isOutput=False)
    kn = nc.declare_dram_parameter("kn", [P, 1], F32, isOutput=False)
    ind = nc.declare_dram_parameter("ind", [P, 2], BF16, isOutput=False)
    ind2 = nc.declare_dram_parameter("ind2", [2, P], F32, isOutput=False)
    one64 = nc.declare_dram_parameter("one64", [1, 64], F32, isOutput=False)
    outp = nc.declare_dram_parameter("out_p", [S, D], F32, isOutput=True)

    with TileContext(nc) as tc:
        with tc.tile_pool(name="big", bufs=1) as big, \
             tc.tile_pool(name="work", bufs=3) as work, \
             tc.tile_pool(name="wbig", bufs=1) as wbig, \
             tc.tile_pool(name="pacc", bufs=2, space="PSUM") as pacc, \
             tc.tile_pool(name="pstr", bufs=4, space="PSUM") as pstr:

            # ---- resident inputs
            xts = big.tile([P, KT, S], BF16)
            nc.sync.dma_start(out=xts[:], in_=xt[:, :, :])
            wqks = big.tile([P, KT, 512], BF16)
            nc.sync.dma_start(out=wqks[:], in_=wqk[:, :, :])
            wvgs = big.tile([P, KT, 260], BF16)
            nc.sync.dma_start(out=wvgs[:], in_=wvg[:, :, :])
            wos = big.tile([64, NH, D], BF16)
            nc.sync.dma_start(out=wos[:], in_=wo[:, :, :])
            rvs = big.tile([P, ST], F32)
            nc.sync.dma_start(out=rvs[:], in_=rv[:, :])
            qns = big.tile([P, 1], F32)
            nc.sync.dma_start(out=qns[:], in_=qn[:, :])
            kns = big.tile([P, 1], F32)
            nc.sync.dma_start(out=kns[:], in_=kn[:, :])
            inds = big.tile([P, 2], BF16)
            nc.sync.dma_start(out=inds[:], in_=ind[:, :])
            ind2s = big.tile([2, P], F32)
            nc.sync.dma_start(out=ind2s[:], in_=ind2[:, :])
            one64s = big.tile([1, 64], F32)
            nc.sync.dma_start(out=one64s[:], in_=one64[:, :])

            ident = big.tile([P, P], F32)
            make_identity(nc, ident[:])
            epsb = big.tile([P, 1], F32)
            nc.vector.memset(epsb[:], EPS)
            eps64 = big.tile([P, 1], F32)
            nc.vector.memset(eps64[:], HD * EPS)

            # ---- resident intermediates
            qkT = big.tile([P, 4, S], BF16)        # mt0,1=q(pair0,1) mt2,3=k
            vbuf = big.tile([P, ST, NH, 65], BF16)  # [sk%128, skt, head, hd+ones]
            nc.vector.memset(vbuf[:], 1.0)
            gnat = big.tile([P, ST, NH], F32)       # sigmoid(r*gate) [s-part]
            gtr = big.tile([1, NH, S], F32)         # gate rows at partition 0
            ssum = big.tile([2, 2, S], F32)         # q sumsq rows
            rqb = big.tile([2, 2, S], F32)          # 1/(8*sqrt(sumsq+64eps))
            rkb = big.tile([P, ST, NH], F32)        # rsqrt per (sk, head)
            obuf = big.tile([64, NH, S], BF16)      # gated attn outT per head

            # ---- phase C: qk projection (+ raw sumsq)
            for mt in range(4):
                for ch in range(NCH):
                    pq = pacc.tile([P, 512], F32, tag="acc")
                    for kt in range(KT):
                        nc.tensor.matmul(
                            pq[:], wqks[:, kt, 128 * mt:128 * mt + 128],
                            xts[:, kt, 512 * ch:512 * ch + 512],
                            start=(kt == 0), stop=(kt == KT - 1))
                    sc = qns if mt < 2 else kns
                    nc.scalar.activation(
                        qkT[:, mt, 512 * ch:512 * ch + 512], pq[:], AF.Copy,
                        scale=sc[:])
                    # squares (raw, pre-norm-weight) for sumsq
                    sq = work.tile([P, 512], BF16, tag="sq")
                    nc.scalar.activation(sq[:], pq[:], AF.Square)
                    if mt < 2:  # q: row-layout sums [2, 512]
                        pr = pstr.tile([2, 512], F32, tag="str")
                        nc.tensor.matmul(pr[:], inds[:], sq[:],
                                         start=True, stop=True)
                        nc.vector.tensor_copy(
                            out=ssum[:, mt, 512 * ch:512 * ch + 512], in_=pr[:])
                    else:  # k: column-layout sums [128, 2] per 128-slice
                        for sl in range(4):
                            pk = pstr.tile([P, 2], F32, tag="str")
                            nc.tensor.matmul(
                                pk[:], sq[:, 128 * sl:128 * sl + 128], inds[:],
                                start=True, stop=True)
                            tmp = work.tile([P, 2], F32, tag="rk_t")
                            nc.scalar.activation(tmp[:], pk[:], AF.Sqrt,
                                                 bias=epsb[:], scale=1.0 / HD)
                            skt = ch * 4 + sl
                            j0 = (mt - 2) * 2
                            nc.vector.reciprocal(
                                rkb[:, skt, j0:j0 + 2], tmp[:])

            # ---- phase D: rq + scale q rows
            for mt in range(2):
                tmp = wbig.tile([2, S], F32, tag="rq_t")
                nc.scalar.activation(tmp[:], ssum[:, mt, :], AF.Sqrt,
                                     bias=eps64[0:2, :], scale=1.0)
                nc.vector.reciprocal(rqb[:, mt, :], tmp[:])
                for ch in range(NCH):
                    pbc = pstr.tile([P, 512], F32, tag="str")
                    nc.tensor.matmul(pbc[:], ind2s[:],
                                     rqb[:, mt, 512 * ch:512 * ch + 512],
                                     start=True, stop=True)
                    nc.vector.tensor_tensor(
                        qkT[:, mt, 512 * ch:512 * ch + 512],
                        qkT[:, mt, 512 * ch:512 * ch + 512], pbc[:],
                        mybir.AluOpType.mult)

            # ---- phase E: v + gate projection
            for t in range(ST):
                pv = pacc.tile([P, 512], F32, tag="acc")
                for kt in range(KT):
                    nc.tensor.matmul(pv[:, 0:260], xts[:, kt, 128 * t:128 * t + 128],
                                     wvgs[:, kt, :],
                                     start=(kt == 0), stop=(kt == KT - 1))
                nc.scalar.activation(vbuf[:, t, :, 0:64], pv[:, 0:256],
                                     AF.Copy, scale=rvs[:, t:t + 1])
                nc.scalar.activation(gnat[:, t, :], pv[:, 256:260],
                                     AF.Sigmoid, scale=rvs[:, t:t + 1])

            # ---- phase F: gate transpose -> per-head rows at partition 0
            for t in range(ST):
                for j in range(NH):
                    pg = pstr.tile([P, 512], F32, tag="str")
                    nc.tensor.transpose(pg[0:1, 0:128], gnat[:, t, j:j + 1],
                                        ident[:])
                    nc.vector.tensor_copy(
                        out=gtr[0:1, j, 128 * t:128 * t + 128],
                        in_=pg[0:1, 0:128])

            # ---- phase G: attention per head
            for j in range(NH):
                a, hp = 64 * (j % 2), j // 2
                for ch in range(NCH):
                    po = pacc.tile([65, 512], F32, tag="acc")
                    for skt in range(ST):
                        ps = pstr.tile([P, 512], F32, tag="str")
                        nc.tensor.matmul(
                            ps[:],
                            qkT[a:a + 64, 2 + hp, 128 * skt:128 * skt + 128],
                            qkT[a:a + 64, hp, 512 * ch:512 * ch + 512],
                            start=True, stop=True)
                        ex = work.tile([P, 512], BF16, tag="ex")
                        nc.scalar.activation(ex[:], ps[:], AF.Exp,
                                             scale=rkb[:, skt, j:j + 1])
                        nc.tensor.matmul(po[:], vbuf[:, skt, j, 0:65], ex[:],
                                         start=(skt == 0), stop=(skt == ST - 1))
                    # finalize: combined = sigmoid(gate)/sums, bcast, apply
                    fin = work.tile([65, 512], F32, tag="fin")
                    nc.vector.reciprocal(fin[64:65, :], po[64:65, :])
                    fr = work.tile([1, 512], F32, tag="fr")
                    nc.sync.dma_start(out=fr[:], in_=fin[64:65, :])
                    cr = work.tile([1, 512], F32, tag="cr")
                    nc.vector.tensor_tensor(
                        cr[:], fr[:], gtr[0:1, j, 512 * ch:512 * ch + 512],
                        mybir.AluOpType.mult)
                    pbc = pstr.tile([P, 512], F32, tag="str")
                    nc.tensor.matmul(pbc[0:64, :], one64s[:], cr[:],
                                     start=True, stop=True)
                    cb = work.tile([64, 512], F32, tag="cb")
                    nc.vector.tensor_copy(out=cb[:], in_=pbc[0:64, :])
                    nc.vector.tensor_tensor(
                        obuf[:, j, 512 * ch:512 * ch + 512], po[0:64, :], cb[:],
                        mybir.AluOpType.mult)

            # ---- phase H: o_proj partial [S, D]
            for t in range(ST):
                for nh in range(2):
                    pp = pacc.tile([P, 512], F32, tag="acc")
                    for j in range(NH):
                        nc.tensor.matmul(
                            pp[:], obuf[:, j, 128 * t:128 * t + 128],
                            wos[:, j, 512 * nh:512 * nh + 512],
                            start=(j == 0), stop=(j == NH - 1))
                    ot = work.tile([P, 512], F32, tag="ot")
                    nc.vector.tensor_copy(out=ot[:], in_=pp[:])
                    nc.sync.dma_start(
                        out=outp[128 * t:128 * t + 128, 512 * nh:512 * nh + 512],
                        in_=ot[:])
    return nc


def _get_program():
    global _NC_CACHE
    if _NC_CACHE is None:
        _NC_CACHE = _build_program()
    return _NC_CACHE


# ----------------------------------------------------------------------------
# host wrapper
# ----------------------------------------------------------------------------

def _prep_inputs(x, prenorm_w, qkv_w, gate_w, o_w, q_norm_w, k_norm_w):
    x = np.asarray(x, np.float32)
    pw = np.asarray(prenorm_w, np.float32)
    qkv_w = np.asarray(qkv_w, np.float32)
    gate_w = np.asarray(gate_w, np.float32)
    o_w = np.asarray(o_w, np.float32)
    qw = qkv_w[0:D] * pw[None, :]
    kw = qkv_w[D:2 * D] * pw[None, :]
    vw = qkv_w[2 * D:3 * D] * pw[None, :]
    gw = gate_w * pw[None, :]

    r = 1.0 / np.sqrt(np.mean(x * x, axis=-1) + EPS)      # [B, S]
    ind = np.zeros((P, 2), BF)
    ind[0:64, 0] = 1
    ind[64:128, 1] = 1
    ind2 = np.zeros((2, P), np.float32)
    ind2[0, 0:64] = 1
    ind2[1, 64:128] = 1
    one64 = np.ones((1, 64), np.float32)
    qn = np.tile(np.asarray(q_norm_w, np.float32), 2)[:, None]
    kn = np.tile(np.asarray(k_norm_w, np.float32), 2)[:, None]

    in_maps = []
    for c in range(8):
        b, hg = c // 4, c % 4
        hsl = slice(256 * hg, 256 * hg + 256)
        xtc = np.ascontiguousarray(
            x[b].T.reshape(KT, P, S).transpose(1, 0, 2)).astype(BF)
        wqk = np.concatenate([qw[hsl], kw[hsl]], 0).T  # [1024, 512]
        wqkc = np.ascontiguousarray(
            wqk.reshape(KT, P, 512).transpose(1, 0, 2)).astype(BF)
        wvg = np.concatenate([vw[hsl], gw[4 * hg:4 * hg + 4]], 0).T  # [1024,260]
        wvgc = np.ascontiguousarray(
            wvg.reshape(KT, P, 260).transpose(1, 0, 2)).astype(BF)
        wo = o_w[:, hsl].T.reshape(NH, 64, D).transpose(1, 0, 2)  # [64, NH, D]
        woc = np.ascontiguousarray(wo).astype(BF)
        rvc = np.ascontiguousarray(r[b].reshape(ST, P).T).astype(np.float32)
        in_maps.append({
            "xt": xtc, "wqk": wqkc, "wvg": wvgc, "wo": woc,
            "rv": rvc, "qn": qn.astype(np.float32),
            "kn": kn.astype(np.float32), "ind": ind, "ind2": ind2,
            "one64": one64,
        })
    return in_maps


def kernel(x, prenorm_w, qkv_w, gate_w, o_w, q_norm_w, k_norm_w):
    from concourse.bass_utils import run_bass_kernel_spmd
    nc = _get_program()
    in_maps = _prep_inputs(x, prenorm_w, qkv_w, gate_w, o_w,
                           q_norm_w, k_norm_w)
    res = run_bass_kernel_spmd(nc, in_maps, list(range(8)))
    outs = [res.results[c]["out_p"] for c in range(8)]
    x = np.asarray(x, np.float32)
    y0 = x[0] + outs[0] + outs[1] + outs[2] + outs[3]
    y1 = x[1] + outs[4] + outs[5] + outs[6] + outs[7]
    return np.stack([y0, y1]).astype(np.float32)
